# revision 22
# baseline (speedup 1.0000x reference)
"""Trainium2 Bass kernel for nn_CustomModelEmbeddingBagGroup (embedding gather-reduce).

Math: the reference's per-bag segment_sum followed by .sum(axis=0) cancels the
bag structure (offsets[0] == 0 makes every index position belong to exactly
one bag), so

    out[t, :] = mult_t * sum_i W_t[eb_input[i], :],   mults = (5, 10, 6).

Device algorithm (8 NeuronCores, histogram by matmul):
  * Vocab rows are split over NCs (250112 rows each); within an NC, row r
    lives at SBUF partition r%128 with column hi = r//128, grouped into 124
    superblocks of 16 hi-bins.
  * Host routes each index to (NC, superblock, partition) — pure
    sharding/reordering — and streams per-batch hi values (bf16).
  * Device builds one-hot rows E[j, hi_bin] = (hi_j == bin) with grouped DVE
    iota-compares (bf16, 2x_1P mode), and PE matmuls with an identity
    stationary matrix accumulate them into PSUM:
        H[p, sb*16 + h] += sum_j I[j, p] * E[j, h]
    i.e. the exact f32 count histogram. 4 batches ride per N=64 matmul in an
    interleaved layout; a DVE tensor_reduce folds the interleave per
    superblock.
  * Readout: fused affine_mul_reduce of H against the host-reshaped tables
    (components = 3 tables x 3 dims) -> [128, 9] partials per NC.
  * Host sums partials over partitions/NCs and applies the multipliers.

Measured on trn2 (8 NCs, axon): ~113 us HW exec, rel err ~4e-5 vs the f32
jax reference (first correct ap_gather design: ~1.78 ms).
"""

import sys

import numpy as np

sys.path.insert(0, "/opt/trn_rl_repo")

N_NC = 8
LO = 128
ROWS_PER_NC = 1954 * 128  # 250112
HI_COLS = 1954
SB = 124
SB_COLS = 16
H_COLS = SB * SB_COLS  # 1984
NUM_EMB = 2_000_000
DIM = 3
N_TABLES = 3
COMPS = N_TABLES * DIM
PAD_VAL = 30000.0
NGROUPS = 1  # compare groups per superblock
MM = 4  # batches per matmul (N = MM*16 = 64)
MULTS = (5.0, 10.0, 6.0)

_kernel_cache: dict[tuple, object] = {}


def _build_device_kernel(g: int):
    """g = batches per compare group (multiple of MM); nbs = NGROUPS*g."""
    from concourse import bacc, mybir, tile

    assert g % MM == 0
    nc = bacc.Bacc("TRN2", target_bir_lowering=False, debug=False)
    nbs = NGROUPS * g
    nb = SB * nbs

    hi_t = nc.dram_tensor("hi_t", [128, nb], mybir.dt.bfloat16, kind="ExternalInput")
    biota = nc.dram_tensor(
        "biota", [128, 4 * SB_COLS * g], mybir.dt.bfloat16, kind="ExternalInput"
    )
    ident = nc.dram_tensor("ident", [128, 128], mybir.dt.bfloat16, kind="ExternalInput")
    w_r = nc.dram_tensor(
        "w_r", [128, COMPS, HI_COLS], mybir.dt.float32, kind="ExternalInput"
    )
    acc = nc.dram_tensor("acc", [128, COMPS], mybir.dt.float32, kind="ExternalOutput")

    with tile.TileContext(nc) as tc:
        with (
            tc.tile_pool(name="con", bufs=1) as con,
            tc.tile_pool(name="eb", bufs=3) as ebp,
            tc.tile_pool(name="ps", bufs=2, space="PSUM") as psp,
        ):
            wt = con.tile([128, COMPS, HI_COLS], mybir.dt.float32)
            hit = con.tile([128, nb], mybir.dt.bfloat16)
            iot = con.tile([128, 4 * SB_COLS * g], mybir.dt.bfloat16)
            idt = con.tile([128, 128], mybir.dt.bfloat16)
            # split the prologue loads across both HWDGE rings so the first
            # compare's inputs (hit, iot) arrive in parallel; w_r is only
            # consumed by the readout at the very end, so it queues behind
            # hit and streams during the main loop.
            nc.scalar.dma_start(out=hit[:], in_=hi_t[:])
            nc.sync.dma_start(out=iot[:], in_=biota[:])
            nc.sync.dma_start(out=idt[:], in_=ident[:])
            nc.scalar.dma_start(out=wt[:], in_=w_r[:])
            hsb = con.tile([128, H_COLS], mybir.dt.float32)

            ch = g // MM  # matmul chunks per superblock
            Q = 4  # superblocks per PSUM tile / compare / fold
            for sbp in range(SB // Q):
                H4 = psp.tile([128, Q * MM * SB_COLS], mybir.dt.float32, space="PSUM")
                col0 = Q * sbp * nbs
                # one compare covers Q superblocks (Q*ch chunks)
                # E[p, c, bin, b] = (bin == hi[p, col0 + c*MM + b])
                ehi = ebp.tile([128, Q * ch, SB_COLS, MM], mybir.dt.bfloat16, tag="ehi")
                hi_b = (
                    hit[:, col0 : col0 + Q * g]
                    .rearrange("p (c b) -> p c b", b=MM)
                    .unsqueeze(2)
                    .broadcast_to([128, Q * ch, SB_COLS, MM])
                )
                io4 = iot[:].rearrange("p (c l b) -> p c l b", l=SB_COLS, b=MM)
                nc.vector.tensor_tensor(
                    out=ehi[:], in0=io4, in1=hi_b, op=mybir.AluOpType.is_equal
                )
                for q in range(Q):
                    hh = H4[:, q * MM * SB_COLS : (q + 1) * MM * SB_COLS]
                    for m in range(ch):
                        rhs = ehi[:, q * ch + m].rearrange("p l b -> p (l b)")
                        nc.tensor.matmul(
                            out=hh,
                            lhsT=idt[:],
                            rhs=rhs,
                            start=(m == 0),
                            stop=(m == ch - 1),
                        )
                # fold all Q sub-histograms: [p, (s h), b] -> reduce X over b
                pv = H4[:].rearrange("p (h b) -> p h b", b=MM)
                nc.vector.tensor_reduce(
                    out=hsb[:, sbp * Q * SB_COLS : (sbp + 1) * Q * SB_COLS],
                    in_=pv,
                    axis=mybir.AxisListType.X,
                    op=mybir.AluOpType.add,
                )

            prod = con.tile([128, HI_COLS], mybir.dt.float32)
            out_t = con.tile([128, COMPS], mybir.dt.float32)
            for c in range(COMPS):
                nc.vector.affine_mul_reduce(
                    out=prod[:],
                    accum_out=out_t[:, c : c + 1],
                    in0=hsb[:, :HI_COLS],
                    in1=wt[:, c],
                    scale=1.0,
                    bias=0.0,
                )
            nc.sync.dma_start(out=acc[:], in_=out_t[:])

    nc.compile()
    _strip_redundant_ldweights(nc)
    return nc


def _strip_redundant_ldweights(nc):
    """All PE weight loads in this kernel load the same identity matrix; the
    lowering still emits one InstLdweights per matmul. Drop every waitless,
    updateless duplicate (any earlier load leaves identical weights in the
    PE array); keep the first load and every sync-carrying one."""
    for b in nc.m.functions[0].blocks:
        insts = b.instructions
        kept_one = False
        drop = []
        for idx, i in enumerate(insts):
            if type(i).__name__ != "InstLdweights":
                continue
            if not kept_one:
                kept_one = True
                continue
            if i.has_wait() or i.has_update():
                continue
            drop.append(idx)
        for idx in reversed(drop):
            del insts[idx]


def _get_device_kernel(g: int):
    if g not in _kernel_cache:
        _kernel_cache[g] = _build_device_kernel(g)
    return _kernel_cache[g]


def _route(eb_input):
    v = np.asarray(eb_input, dtype=np.int64)
    n = v // ROWS_PER_NC
    r = v - n * ROWS_PER_NC
    lo = r & 127  # partition
    hi = r >> 7
    sb = hi // SB_COLS
    hirel = (hi % SB_COLS).astype(np.float32)
    cell = (n * SB + sb) * 128 + lo  # 16384 cells
    return cell, hirel


def _prepare_inputs(eb_input, g):
    import ml_dtypes

    nbs = NGROUPS * g
    nb = SB * nbs
    cell, hirel = _route(eb_input)
    order = np.argsort(cell, kind="stable")
    cell_s = cell[order]
    hirel_s = hirel[order]
    counts = np.bincount(cell, minlength=N_NC * SB * 128)
    offs = np.zeros(N_NC * SB * 128 + 1, np.int64)
    np.cumsum(counts, out=offs[1:])
    rank = np.arange(len(cell_s)) - offs[cell_s]  # position within cell

    # destination flat position in a per-NC [128, nb] array:
    #   partition lo, column sb*nbs + rank
    ncid = cell_s // (SB * 128)
    sbid = (cell_s >> 7) % SB
    loid = cell_s & 127
    flat = loid * nb + sbid * nbs + rank

    bio = np.broadcast_to(
        np.tile(np.repeat(np.arange(SB_COLS, dtype=np.float32), MM), 4 * (g // MM)),
        (128, 4 * SB_COLS * g),
    ).astype(ml_dtypes.bfloat16)
    identity = np.eye(128, dtype=ml_dtypes.bfloat16)
    in_maps = []
    for n in range(N_NC):
        sel = ncid == n
        hi_arr = np.full(128 * nb, PAD_VAL, np.float32)
        hi_arr[flat[sel]] = hirel_s[sel]
        in_maps.append(
            {
                "hi_t": hi_arr.reshape(128, nb).astype(ml_dtypes.bfloat16),
                "biota": bio,
                "ident": identity,
            }
        )
    return in_maps


def _prepare_tables(W0, W1, W2):
    Ws = [np.asarray(w, dtype=np.float32) for w in (W0, W1, W2)]
    per_nc = []
    for n in range(N_NC):
        base = n * ROWS_PER_NC
        nrows = min(ROWS_PER_NC, max(0, NUM_EMB - base))
        wr = np.zeros((128, COMPS, HI_COLS), np.float32)
        if nrows > 0:
            nhi = -(-nrows // 128)
            for t in range(N_TABLES):
                blk = np.zeros((nhi * 128, DIM), np.float32)
                blk[:nrows] = Ws[t][base : base + nrows]
                wr[:, 3 * t : 3 * t + 3, :nhi] = blk.reshape(nhi, 128, DIM).transpose(
                    1, 2, 0
                )
        per_nc.append(wr)
    return per_nc


NBS_CAP = 384  # beyond this the E tiles would pressure SBUF; split instead


def run(eb_input, eb_offset, W0, W1, W2, trace=False, **spmd_kwargs):
    from concourse.bass_utils import run_bass_kernel_spmd

    cell_probe, _ = _route(eb_input)
    counts_probe = np.bincount(cell_probe, minlength=N_NC * SB * 128)
    need = -(-int(counts_probe.max()) // MM) * MM
    if need > NBS_CAP:
        # heavily skewed input: process interleaved slices and sum (each
        # slice has proportionally smaller per-cell maxima)
        nsplit = -(-need // NBS_CAP)
        total = None
        res = None
        for si in range(nsplit):
            out_i, res = run(
                np.asarray(eb_input)[si::nsplit], eb_offset, W0, W1, W2,
                trace=trace, **spmd_kwargs,
            )
            total = out_i if total is None else total + out_i
        return total.astype(np.float32), res

    cell, _ = _route(eb_input)
    counts = np.bincount(cell, minlength=N_NC * SB * 128)
    g = -(-int(counts.max()) // MM) * MM  # batches per sb, multiple of MM

    nc = _get_device_kernel(g)
    in_maps = _prepare_inputs(eb_input, g)
    tables = _prepare_tables(W0, W1, W2)
    for n in range(N_NC):
        in_maps[n]["w_r"] = tables[n]
    res = run_bass_kernel_spmd(
        nc, in_maps, core_ids=list(range(N_NC)), trace=trace, **spmd_kwargs
    )
    totals = np.zeros((N_TABLES, DIM), np.float64)
    for n in range(N_NC):
        a = np.asarray(res.results[n]["acc"], dtype=np.float64)
        for t in range(N_TABLES):
            for d in range(DIM):
                totals[t, d] += a[:, 3 * t + d].sum()
    out = np.stack([MULTS[t] * totals[t] for t in range(N_TABLES)]).astype(np.float32)
    return out, res


def kernel(eb_input, eb_offset, W0, W1, W2):
    out, _ = run(eb_input, eb_offset, W0, W1, W2, trace=False)
    return out
